# revision 2
# baseline (speedup 1.0000x reference)
"""AgreementRouting (CapsNet dynamic routing) Trainium2 Bass kernel.

Full input [256, 1152, 10, 16] f32 -> v [256, 10, 16] f32.
Data-parallel over batch: 32 samples per core on 8 cores.

Per-core plan (memory-regime):
  - One f32 HBM read of the 23.6MB shard; convert to resident bf16 SBUF copy
    (11.8MB) while accumulating s0 = 0.1*sum_i X via ones-matmul on PE.
  - 3 routing iterations fully on-chip:
      einsum1 (agreement dot over d): bf16 mul (GpSimd/DVE) + grouped reduce (DVE)
      softmax over o: exp on ACT, sums/recip/mul on DVE
      einsum2 (weighted sum over i): PE matmuls, c as stationary lhsT
      squash + v feedback: small DVE/ACT ops, PE ones-matmul colsum +
      outer-product broadcast of v to all 128 partitions.
"""

import numpy as np

import concourse.bacc as bacc
import concourse.bass as bass
import concourse.mybir as mybir
import concourse.tile as tile
from concourse.bass import AP

NCORES = 8
B = 256
S = B // NCORES          # 32 samples per core
I = 1152
O = 10
D = 16
OD = O * D               # 160
P = 128
NCH = I // P             # 9 chunks of 128 input caps
GS = 8                   # sample group size for batched softmax/einsum1
NG = S // GS             # 4 groups
N_ITER = 3

F32 = mybir.dt.float32
BF16 = mybir.dt.bfloat16
AX = mybir.AxisListType.X
AF = mybir.ActivationFunctionType
ALU = mybir.AluOpType


def _rep_mid(a, n, at=1):
    """Insert a step-0 (broadcast) dim of size n into free dims at position `at`
    (1 == right after the partition dim)."""
    ap = [list(e) for e in a.ap]
    ap = ap[:at] + [[0, n]] + ap[at:]
    return AP(a.tensor, a.offset, ap)


def _rep_last(a, n):
    """Append trailing step-0 (broadcast) dim of size n."""
    ap = [list(e) for e in a.ap] + [[0, n]]
    return AP(a.tensor, a.offset, ap)


def _build():
    nc = bacc.Bacc(None, target_bir_lowering=False)
    # f32 input viewed as bf16 pairs: [..., 1::2] is the high half = truncated bf16
    x = nc.dram_tensor("x", [S, I, O, 2 * D], BF16, kind="ExternalInput")
    vout = nc.dram_tensor("v", [S, O, D], F32, kind="ExternalOutput")

    with tile.TileContext(nc) as tc:
        with (
            tc.tile_pool(name="big", bufs=1) as big,
            tc.tile_pool(name="stage", bufs=2) as stagep,  # (unused; kept for layout)
            tc.tile_pool(name="work", bufs=2) as workp,
            tc.tile_pool(name="small", bufs=3) as smallp,
            tc.tile_pool(name="const", bufs=1) as constp,
            tc.tile_pool(name="ps_s0", bufs=2, space="PSUM") as ps_s0,
            tc.tile_pool(name="ps_s", bufs=3, space="PSUM") as ps_sp,
            tc.tile_pool(name="ps_vb", bufs=1, space="PSUM") as ps_vbp,
            tc.tile_pool(name="ps_bc", bufs=2, space="PSUM") as ps_bcp,
        ):
            # ---- persistent state ----
            Xb = big.tile([P, S * NCH * OD], BF16)      # bf16 input, (s, ch, o, d) free order
            vb_all = big.tile([P, S * OD], BF16)        # v broadcast to all partitions, (s, o, d)
            b_cum = big.tile([P, S * NCH * O], F32)     # routing logits, (s, ch, o)
            c_t = big.tile([P, S * NCH * O], BF16)      # softmax coeffs, (s, ch, o)

            # ---- constants ----
            mask = constp.tile([O, OD], F32)            # block-diagonal o/od mask
            ones128 = constp.tile([P, 1], BF16)
            ones10 = constp.tile([O, 1], F32)
            ones1 = constp.tile([1, P], F32)
            ones16 = constp.tile([1, D], F32)
            nc.vector.memset(ones16[:], 1.0)
            nc.vector.memset(mask[:], 0.0)
            for o in range(O):
                # engines can't start at partition o>0, but DMA can
                nc.sync.dma_start(mask[o : o + 1, o * D : (o + 1) * D], ones16[:])
            nc.vector.memset(ones128[:], 1.0)
            nc.vector.memset(ones10[:], 1.0)
            nc.vector.memset(ones1[:], 1.0)

            def squash_scale(l2, tag):
                """l2: [p, n] f32 -> scale = sqrt(l2)/(1+l2), shape [p, n]."""
                pdim = l2.shape[0]
                n = l2.shape[1]
                rt = smallp.tile([pdim, n], F32, tag=tag + "rt")
                nc.scalar.activation(rt[:], l2[:], AF.Sqrt)
                den = smallp.tile([pdim, n], F32, tag=tag + "dn")
                nc.vector.tensor_single_scalar(den[:], l2[:], 1.0, op=ALU.add)
                rden = smallp.tile([pdim, n], F32, tag=tag + "rd")
                nc.vector.reciprocal(rden[:], den[:])
                sc = smallp.tile([pdim, n], F32, tag=tag + "sc")
                nc.vector.tensor_mul(sc[:], rt[:], rden[:])
                return sc

            # PE instructions may carry only ONE semaphore wait, so every matmul
            # dependency must be either DMA or DVE-produced (single monotone sem).
            # Warmup matmul: makes PE observe the DVE constants tick once.
            warm = ps_bcp.tile([P, OD], F32, tag="bc")
            nc.tensor.matmul(warm[:, :1], ones1[:], ones16[:, :1], start=True, stop=True)

            def broadcast_v(v_sb, b):
                """v_sb [1, OD] f32 SBUF -> vb_all[:, b*OD:(b+1)*OD] bf16 on all partitions."""
                bc = ps_bcp.tile([P, OD], F32, tag="bc")
                nc.tensor.matmul(bc[:], ones1[:], v_sb[:], start=True, stop=True)
                nc.vector.tensor_copy(vb_all[:, b * OD : (b + 1) * OD], bc[:])

            # ---- phase 0: load (bf16 truncation via strided DMA) + s0 ----
            for b in range(S):
                ps0 = ps_s0.tile([1, OD], F32)
                for ch in range(NCH):
                    base = (b * NCH + ch) * OD
                    nc.sync.dma_start(
                        Xb[:, base : base + OD].rearrange("p (o d) -> p o d", o=O),
                        x[b, ch * P : (ch + 1) * P].rearrange(
                            "p o (d t) -> p o d t", t=2
                        )[:, :, :, 1],
                    )
                    nc.tensor.matmul(
                        ps0[:],
                        ones128[:],
                        Xb[:, base : base + OD],
                        start=(ch == 0),
                        stop=(ch == NCH - 1),
                    )
                # squash(s0) on one partition row (DVE reads PSUM so the slot
                # release stays on the DVE semaphore)
                s0 = smallp.tile([1, OD], F32, tag="s0")
                nc.vector.tensor_single_scalar(s0[:], ps0[:], 0.1, op=ALU.mult)
                sq = smallp.tile([1, OD], F32, tag="sq0")
                nc.vector.tensor_mul(sq[:], s0[:], s0[:])
                l2 = smallp.tile([1, O], F32, tag="l20")
                nc.vector.reduce_sum(
                    l2[:], sq[:].rearrange("p (o d) -> p o d", d=D), axis=AX
                )
                sc = squash_scale(l2, "p0")
                v0 = smallp.tile([1, OD], F32, tag="v0")
                nc.vector.tensor_mul(
                    v0[:].rearrange("p (o d) -> p o d", d=D),
                    s0[:].rearrange("p (o d) -> p o d", d=D),
                    _rep_last(sc[:], D),
                )
                broadcast_v(v0, b)

            # ---- routing iterations ----
            for k in range(N_ITER):
                last = k == N_ITER - 1
                for g in range(NG):
                    gb = g * GS
                    # einsum1: b_inc[s,i,o] = sum_d X[s,i,o,d] * v[s,o,d]
                    tmp = workp.tile([P, GS * NCH * OD], BF16, tag="tmp")
                    xs = Xb[:, gb * NCH * OD : (gb + GS) * NCH * OD].rearrange(
                        "p (s c f) -> p s c f", s=GS, c=NCH
                    )
                    vb = vb_all[:, gb * OD : (gb + GS) * OD].rearrange(
                        "p (s f) -> p s f", s=GS
                    )
                    vb_rep = _rep_mid(vb, NCH, at=2)  # [P, GS, NCH, OD] step-0 on NCH
                    mul_eng = nc.gpsimd if (g % 2 == 0) else nc.vector
                    mul_eng.tensor_mul(
                        tmp[:].rearrange("p (s c f) -> p s c f", s=GS, c=NCH),
                        xs,
                        vb_rep,
                    )
                    # grouped reduce over d
                    if k == 0:
                        red_out = b_cum[:, gb * NCH * O : (gb + GS) * NCH * O]
                    else:
                        binc = workp.tile([P, GS * NCH * O], F32, tag="binc")
                        red_out = binc[:]
                    nc.vector.reduce_sum(
                        red_out,
                        tmp[:].rearrange("p (m d2) -> p m d2", d2=D),
                        axis=AX,
                    )
                    if k != 0:
                        nc.vector.tensor_add(
                            b_cum[:, gb * NCH * O : (gb + GS) * NCH * O],
                            b_cum[:, gb * NCH * O : (gb + GS) * NCH * O],
                            red_out,
                        )
                    # softmax over o (free groups of 10)
                    bsl = b_cum[:, gb * NCH * O : (gb + GS) * NCH * O]
                    e_g = workp.tile([P, GS * NCH * O], F32, tag="eg")
                    nc.scalar.activation(e_g[:], bsl, AF.Exp)
                    rs = smallp.tile([P, GS * NCH], F32, tag="rs")
                    nc.vector.reduce_sum(
                        rs[:], e_g[:].rearrange("p (m o) -> p m o", o=O), axis=AX
                    )
                    rr = smallp.tile([P, GS * NCH], F32, tag="rr")
                    nc.vector.reciprocal(rr[:], rs[:])
                    nc.vector.tensor_mul(
                        c_t[:, gb * NCH * O : (gb + GS) * NCH * O].rearrange(
                            "p (m o) -> p m o", o=O
                        ),
                        e_g[:].rearrange("p (m o) -> p m o", o=O),
                        _rep_last(rr[:], O),
                    )
                    # einsum2 + squash per sample in this group
                    for b in range(gb, gb + GS):
                        ps_s = ps_sp.tile([O, OD], F32)
                        for ch in range(NCH):
                            cb = (b * NCH + ch) * O
                            xb = (b * NCH + ch) * OD
                            nc.tensor.matmul(
                                ps_s[:],
                                c_t[:, cb : cb + O],
                                Xb[:, xb : xb + OD],
                                start=(ch == 0),
                                stop=(ch == NCH - 1),
                            )
                        # masked squash: only diagonal o blocks of ps_s are real
                        masked = smallp.tile([O, OD], F32, tag="mk")
                        nc.vector.tensor_mul(masked[:], ps_s[:], mask[:])
                        sqs = smallp.tile([O, OD], F32, tag="sqs")
                        l2b = smallp.tile([O, 1], F32, tag="l2b")
                        nc.scalar.activation(
                            sqs[:], masked[:], AF.Square, accum_out=l2b[:]
                        )
                        scb = squash_scale(l2b, "it")
                        v_full = smallp.tile([O, OD], F32, tag="vf")
                        nc.vector.tensor_single_scalar(
                            v_full[:], masked[:], scb[:], op=ALU.mult
                        )
                        # collapse rows: vb[od] = sum_o v_full[o, od] (one nonzero each)
                        ps_vb = ps_vbp.tile([1, OD], F32)
                        nc.tensor.matmul(
                            ps_vb[:], ones10[:], v_full[:], start=True, stop=True
                        )
                        v_sb = smallp.tile([1, OD], F32, tag="vsb")
                        nc.vector.tensor_copy(v_sb[:], ps_vb[:])
                        if last:
                            nc.sync.dma_start(
                                vout[b : b + 1].rearrange("b o d -> b (o d)"), v_sb[:]
                            )
                        else:
                            broadcast_v(v_sb, b)

    nc.compile()
    return nc


_cached = {}


def _get_nc():
    if "nc" not in _cached:
        _cached["nc"] = _build()
    return _cached["nc"]


def kernel(input, _trace=False):
    from concourse.bass_utils import run_bass_kernel_spmd

    import ml_dtypes

    input = np.ascontiguousarray(np.asarray(input, dtype=np.float32))
    assert input.shape == (B, I, O, D)
    nc = _get_nc()
    xview = input.view(ml_dtypes.bfloat16).reshape(B, I, O, 2 * D)
    in_maps = [
        {"x": np.ascontiguousarray(xview[c * S : (c + 1) * S])} for c in range(NCORES)
    ]
    res = run_bass_kernel_spmd(
        nc, in_maps, core_ids=list(range(NCORES)), trace=_trace
    )
    out = np.concatenate([r["v"] for r in res.results], axis=0)
    if _trace:
        kernel.last_exec_time_ns = res.exec_time_ns
        kernel.last_res = res
    return out.astype(np.float32)


kernel.last_exec_time_ns = None



# revision 12
# speedup vs baseline: 10.4801x; 10.4801x over previous
"""AgreementRouting (CapsNet dynamic routing) Trainium2 Bass kernel, v2.

Full input [256, 1152, 10, 16] f32 -> v [256, 10, 16] f32.
Data-parallel over batch: 32 samples per core on 8 cores.

Per-core plan (memory-regime):
  - Load: per-sample HWDGE DMA of contiguous f32 (i permuted so each
    partition reads ONE contiguous 5760B run: i = 9*p + j), staged in SBUF
    f32, converted to resident bf16 Xb by DVE 2x copies.  This replaces the
    old 2-byte-strided bf16 extraction that generated 5.9M descriptors.
  - Phase -1 (s0): einsum2 with constant c=0.1 stationary, col-tiled
    4 samples/PSUM tile, batched squash, matmul collapse+broadcast of v.
  - 3 routing iterations fully on-chip, batches of 4 samples:
      einsum1: bf16 mul (DVE 2x / GpSimd) + bf16 pair-tree reduction over d
               (TT adds at 2x beat tensor_reduce's 1x mode); 2 of 8 batches
               run on GpSimd to unload DVE (the bottleneck engine)
      softmax over o: ACT Exp + DVE sum/recip/mul
      einsum2: PE matmuls, c stationary, 4 samples col-tiled per PSUM tile
      squash: batched over 4 samples on [128,*] tiles; sqrt via exp(.5*ln)
              so ACT stays on the natural_log_exp table set (no reloads)
      v feedback: one [128,128]-ones matmul per sample does collapse AND
              broadcast-to-all-partitions in one shot.
  - All PE dependencies are DVE-produced (PE instrs carry one sem wait).
"""

import numpy as np

import concourse.bacc as bacc
import concourse.bass as bass
import concourse.mybir as mybir
import concourse.tile as tile
from concourse.bass import AP

NCORES = 8
B = 256
S = B // NCORES          # 32 samples per core
I = 1152
O = 10
D = 16
OD = O * D               # 160
P = 128
NJ = I // P              # 9: i = 9*p + j
SW = NJ * OD             # 1440 elems per sample per partition
TB = 4                   # batch: einsum1 group == einsum2 col-tiled samples
NB = S // TB             # 8 batches
GPS_BATCHES = (3, 7)     # einsum1 batches offloaded to GpSimd
N_ITER = 3

F32 = mybir.dt.float32
BF16 = mybir.dt.bfloat16
AX = mybir.AxisListType.X
AF = mybir.ActivationFunctionType
ALU = mybir.AluOpType


def _rep_mid(a, n, at=1):
    """Insert a step-0 (broadcast) dim of size n into free dims at position `at`."""
    ap = [list(e) for e in a.ap]
    ap = ap[:at] + [[0, n]] + ap[at:]
    return AP(a.tensor, a.offset, ap)


def _rep_last(a, n):
    ap = [list(e) for e in a.ap] + [[0, n]]
    return AP(a.tensor, a.offset, ap)


def _build():
    nc = bacc.Bacc(None, target_bir_lowering=False)
    x = nc.dram_tensor("x", [S, I, O, D], F32, kind="ExternalInput")
    vout = nc.dram_tensor("v", [S, O, D], F32, kind="ExternalOutput")

    with tile.TileContext(nc) as tc:
        with (
            tc.tile_pool(name="big", bufs=1) as big,
            tc.tile_pool(name="stage", bufs=3) as stagep,
            tc.tile_pool(name="e1", bufs=1) as e1p,      # einsum1 scratch
            tc.tile_pool(name="bi", bufs=2) as bip,
            tc.tile_pool(name="work", bufs=2) as workp,
            tc.tile_pool(name="small", bufs=3) as smallp,
            tc.tile_pool(name="const", bufs=1) as constp,
            tc.tile_pool(name="ps_s", bufs=3, space="PSUM") as ps_sp,
            tc.tile_pool(name="ps_bc", bufs=2, space="PSUM") as ps_bcp,
            tc.tile_pool(name="ps_v", bufs=2, space="PSUM") as ps_vp,
            tc.tile_pool(name="ps_w", bufs=1, space="PSUM") as ps_wp,
        ):
            # ---- persistent state ----
            Xb = big.tile([P, S * SW], BF16)          # bf16 input, (s, j, o, d)
            vb_all = big.tile([P, S * OD], BF16)      # v bcast to all partitions
            b_cum = big.tile([P, S * NJ * O], F32)    # routing logits (s, j, o)
            c_t = big.tile([P, S * NJ * O], BF16)     # softmax coeffs (s, j, o)

            # ---- constants ----
            mask128 = constp.tile([P, OD], F32)       # 4 strips of block-diag o/od
            ones16 = constp.tile([1, D], F32)
            onesrow = constp.tile([32, P], BF16)      # src for A strips
            A = constp.tile([P, TB * P], BF16)        # A[:, t*P:(t+1)*P]: ones on strip t
            c01 = constp.tile([P, O], BF16)           # uniform routing c = 0.1

            nc.vector.memset(ones16[:], 1.0)
            nc.vector.memset(mask128[:], 0.0)
            for t in range(TB):
                for o in range(O):
                    # engines can't write at partition offsets; DMA can
                    nc.sync.dma_start(
                        mask128[32 * t + o : 32 * t + o + 1, o * D : (o + 1) * D],
                        ones16[:],
                    )
            nc.vector.memset(onesrow[:], 1.0)
            nc.vector.memset(A[:], 0.0)
            for t in range(TB):
                nc.sync.dma_start(
                    A[32 * t : 32 * (t + 1), t * P : (t + 1) * P], onesrow[:]
                )
            nc.vector.memset(c01[:], 0.1)
            bias_one = constp.tile([P, 1], F32)
            nc.vector.memset(bias_one[:], 1.0)
            bias_eps = constp.tile([P, 1], F32)
            nc.vector.memset(bias_eps[:], 1e-30)

            # PE warmup so PE observes the DVE-const tick once
            warm = ps_wp.tile([1, O], F32, tag="warm")
            nc.tensor.matmul(warm[:], A[:, :1], c01[:], start=True, stop=True)

            def load_batch(q):
                for t in range(TB):
                    b = TB * q + t
                    st = stagep.tile([P, SW], F32)
                    nc.sync.dma_start(
                        st[:],
                        x[b].rearrange("(p j) o d -> p (j o d)", p=P),
                    )
                    nc.vector.tensor_copy(Xb[:, b * SW : (b + 1) * SW], st[:])

            def einsum1_batch(q, k):
                """b_cum[s,i,o] += sum_d X * v for samples 4q..4q+3."""
                on_gps = q in GPS_BATCHES
                eng = nc.gpsimd if on_gps else nc.vector
                sfx = "g" if on_gps else "d"
                s0 = TB * q
                M = TB * NJ * O          # 360 d-groups
                tmp = e1p.tile([P, TB * SW], BF16, tag="tmp" + sfx)
                xs = Xb[:, s0 * SW : (s0 + TB) * SW].rearrange(
                    "p (s j f) -> p s j f", s=TB, j=NJ
                )
                vb = vb_all[:, s0 * OD : (s0 + TB) * OD].rearrange(
                    "p (s f) -> p s f", s=TB
                )
                eng.tensor_mul(
                    tmp[:].rearrange("p (s j f) -> p s j f", s=TB, j=NJ),
                    xs,
                    _rep_mid(vb, NJ, at=2),
                )
                tv = tmp[:].rearrange("p (m e) -> p m e", e=D)
                t1 = e1p.tile([P, M * 8], BF16, tag="t1" + sfx)
                t1v = t1[:].rearrange("p (m e) -> p m e", e=8)
                eng.tensor_add(t1v, tv[:, :, 0:8], tv[:, :, 8:16])
                t2 = e1p.tile([P, M * 4], BF16, tag="t2" + sfx)
                t2v = t2[:].rearrange("p (m e) -> p m e", e=4)
                eng.tensor_add(t2v, t1v[:, :, 0:4], t1v[:, :, 4:8])
                t3 = e1p.tile([P, M * 2], BF16, tag="t3" + sfx)
                t3v = t3[:].rearrange("p (m e) -> p m e", e=2)
                eng.tensor_add(t3v, t2v[:, :, 0:2], t2v[:, :, 2:4])
                bsl = b_cum[:, s0 * NJ * O : (s0 + TB) * NJ * O]
                if k == 0:
                    eng.tensor_add(
                        bsl.rearrange("p (m e) -> p m e", e=1),
                        t3v[:, :, 0:1],
                        t3v[:, :, 1:2],
                    )
                else:
                    binc = bip.tile([P, M], F32, tag="bi" + sfx)
                    eng.tensor_add(
                        binc[:].rearrange("p (m e) -> p m e", e=1),
                        t3v[:, :, 0:1],
                        t3v[:, :, 1:2],
                    )
                    nc.gpsimd.tensor_add(bsl, bsl, binc[:])

            def softmax_batch(q):
                s0 = TB * q
                M = TB * NJ              # 36 softmax rows per partition
                bsl = b_cum[:, s0 * NJ * O : (s0 + TB) * NJ * O]
                e_g = workp.tile([P, TB * NJ * O], F32, tag="eg")
                nc.scalar.activation(e_g[:], bsl, AF.Exp)
                rs = smallp.tile([P, M], F32, tag="rs")
                nc.vector.reduce_sum(
                    rs[:], e_g[:].rearrange("p (m o) -> p m o", o=O), axis=AX
                )
                rr = smallp.tile([P, M], F32, tag="rr")
                nc.vector.reciprocal(rr[:], rs[:])
                nc.vector.tensor_mul(
                    c_t[:, s0 * NJ * O : (s0 + TB) * NJ * O].rearrange(
                        "p (m o) -> p m o", o=O
                    ),
                    e_g[:].rearrange("p (m o) -> p m o", o=O),
                    _rep_last(rr[:], O),
                )

            def einsum2_batch(q, stationary):
                """4 col-tiled samples' s = sum_i c[i,o] X[i,od] -> ps [128,160]."""
                ps = ps_sp.tile([P, OD], F32)
                for j in range(NJ):
                    for t in range(TB):
                        b = TB * q + t
                        if stationary is None:
                            lhsT = c_t[:, (b * NJ + j) * O : (b * NJ + j + 1) * O]
                        else:
                            lhsT = stationary[:]
                        nc.tensor.matmul(
                            ps[32 * t : 32 * t + O, :],
                            lhsT,
                            Xb[:, (b * SW + j * OD) : (b * SW + (j + 1) * OD)],
                            start=(j == 0),
                            stop=(j == NJ - 1),
                            tile_position=(0, 32 * t),
                        )
                return ps

            def squash_batch(q, ps):
                """Batched masked squash of ps -> v_full bf16 [128, 160]."""
                masked = smallp.tile([P, OD], F32, tag="mk")
                nc.vector.tensor_mul(masked[:], ps[:], mask128[:])
                sq = smallp.tile([P, OD], BF16, tag="sq")
                l2 = smallp.tile([P, 1], F32, tag="l2")
                nc.scalar.activation(sq[:], masked[:], AF.Square, accum_out=l2[:])
                den = smallp.tile([P, 1], F32, tag="dn")
                nc.scalar.activation(den[:], l2[:], AF.Identity, bias=bias_one[:])
                rden = smallp.tile([P, 1], F32, tag="rd")
                nc.vector.reciprocal(rden[:], den[:])
                lnl2 = smallp.tile([P, 1], F32, tag="lg")
                nc.scalar.activation(lnl2[:], l2[:], AF.Ln, bias=bias_eps[:])
                rt = smallp.tile([P, 1], F32, tag="rt")
                nc.scalar.activation(rt[:], lnl2[:], AF.Exp, scale=0.5)
                sc = smallp.tile([P, 1], F32, tag="sc")
                nc.vector.tensor_mul(sc[:], rt[:], rden[:])
                v_full = smallp.tile([P, OD], BF16, tag="vf")
                nc.vector.tensor_single_scalar(v_full[:], masked[:], sc[:], op=ALU.mult)
                return v_full

            def broadcast_batch(q, v_full):
                """v per sample -> all partitions of vb_all (collapse+bcast matmul)."""
                for half in range(2):
                    bc = ps_bcp.tile([P, 2 * OD], F32, tag="bc")
                    for u in range(2):
                        t = 2 * half + u
                        nc.tensor.matmul(
                            bc[:, u * OD : (u + 1) * OD],
                            A[:, t * P : (t + 1) * P],
                            v_full[:],
                            start=True,
                            stop=True,
                        )
                    nc.vector.tensor_copy(
                        vb_all[
                            :,
                            (TB * q + 2 * half) * OD : (TB * q + 2 * half + 2) * OD,
                        ],
                        bc[:],
                    )

            def output_batch(q, v_full):
                for t in range(TB):
                    b = TB * q + t
                    psv = ps_vp.tile([1, OD], F32, tag="pv")
                    nc.tensor.matmul(
                        psv[:], A[:, t * P : t * P + 1], v_full[:],
                        start=True, stop=True,
                    )
                    v_sb = smallp.tile([1, OD], F32, tag="vo")
                    nc.vector.tensor_copy(v_sb[:], psv[:])
                    nc.sync.dma_start(
                        vout[b : b + 1].rearrange("b o d -> b (o d)"), v_sb[:]
                    )

            # ---- phase -1 (load + s0) merged with iteration 0 ----
            for q in range(NB):
                load_batch(q)
                ps = einsum2_batch(q, c01)
                vf = squash_batch(q, ps)
                broadcast_batch(q, vf)
                einsum1_batch(q, 0)
                softmax_batch(q)
                ps = einsum2_batch(q, None)
                vf = squash_batch(q, ps)
                broadcast_batch(q, vf)

            # ---- iterations 1..2 ----
            for k in range(1, N_ITER):
                last = k == N_ITER - 1
                for q in range(NB):
                    einsum1_batch(q, k)
                    softmax_batch(q)
                    ps = einsum2_batch(q, None)
                    vf = squash_batch(q, ps)
                    if last:
                        output_batch(q, vf)
                    else:
                        broadcast_batch(q, vf)

    nc.compile()
    return nc


_cached = {}


def _get_nc():
    if "nc" not in _cached:
        _cached["nc"] = _build()
    return _cached["nc"]


def kernel(input, _trace=False):
    from concourse.bass_utils import run_bass_kernel_spmd

    input = np.ascontiguousarray(np.asarray(input, dtype=np.float32))
    assert input.shape == (B, I, O, D)
    nc = _get_nc()
    in_maps = [{"x": input[c * S : (c + 1) * S]} for c in range(NCORES)]
    res = run_bass_kernel_spmd(
        nc, in_maps, core_ids=list(range(NCORES)), trace=_trace
    )
    out = np.concatenate([r["v"] for r in res.results], axis=0)
    if _trace:
        kernel.last_exec_time_ns = res.exec_time_ns
        kernel.last_res = res
    return out.astype(np.float32)


kernel.last_exec_time_ns = None


# revision 16
# speedup vs baseline: 11.8500x; 1.1307x over previous
"""AgreementRouting (CapsNet dynamic routing) Trainium2 Bass kernel, v2.

Full input [256, 1152, 10, 16] f32 -> v [256, 10, 16] f32.
Data-parallel over batch: 32 samples per core on 8 cores.

Per-core plan (memory-regime):
  - Load: per-sample HWDGE DMA of contiguous f32 (i permuted so each
    partition reads ONE contiguous 5760B run: i = 9*p + j), staged in SBUF
    f32, converted to resident bf16 Xb by DVE 2x copies.  This replaces the
    old 2-byte-strided bf16 extraction that generated 5.9M descriptors.
  - Phase -1 (s0): einsum2 with constant c=0.1 stationary, col-tiled
    4 samples/PSUM tile, batched squash, matmul collapse+broadcast of v.
  - 3 routing iterations fully on-chip, batches of 4 samples:
      einsum1: bf16 mul (DVE 2x / GpSimd) + bf16 pair-tree reduction over d
               (TT adds at 2x beat tensor_reduce's 1x mode); 2 of 8 batches
               run on GpSimd to unload DVE (the bottleneck engine)
      softmax over o: ACT Exp + DVE sum/recip/mul
      einsum2: PE matmuls, c stationary, 4 samples col-tiled per PSUM tile
      squash: batched over 4 samples on [128,*] tiles; sqrt via exp(.5*ln)
              so ACT stays on the natural_log_exp table set (no reloads)
      v feedback: one [128,128]-ones matmul per sample does collapse AND
              broadcast-to-all-partitions in one shot.
  - All PE dependencies are DVE-produced (PE instrs carry one sem wait).
"""

import numpy as np

import concourse.bacc as bacc
import concourse.bass as bass
import concourse.mybir as mybir
import concourse.tile as tile
from concourse.bass import AP

NCORES = 8
B = 256
S = B // NCORES          # 32 samples per core
I = 1152
O = 10
D = 16
OD = O * D               # 160
P = 128
NJ = I // P              # 9: i = 9*p + j
SW = NJ * OD             # 1440 elems per sample per partition
TB = 4                   # batch: einsum1 group == einsum2 col-tiled samples
NB = S // TB             # 8 batches
GPS_BATCHES = (3, 7)     # einsum1 batches offloaded to GpSimd
N_ITER = 3

F32 = mybir.dt.float32
BF16 = mybir.dt.bfloat16
AX = mybir.AxisListType.X
AF = mybir.ActivationFunctionType
ALU = mybir.AluOpType


def _rep_mid(a, n, at=1):
    """Insert a step-0 (broadcast) dim of size n into free dims at position `at`."""
    ap = [list(e) for e in a.ap]
    ap = ap[:at] + [[0, n]] + ap[at:]
    return AP(a.tensor, a.offset, ap)


def _rep_last(a, n):
    ap = [list(e) for e in a.ap] + [[0, n]]
    return AP(a.tensor, a.offset, ap)


def _build():
    nc = bacc.Bacc(None, target_bir_lowering=False)
    x = nc.dram_tensor("x", [S, I, O, D], F32, kind="ExternalInput")
    vout = nc.dram_tensor("v", [S, O, D], F32, kind="ExternalOutput")

    with tile.TileContext(nc) as tc:
        with (
            tc.tile_pool(name="big", bufs=1) as big,
            tc.tile_pool(name="stage", bufs=3) as stagep,
            tc.tile_pool(name="e1", bufs=1) as e1p,      # einsum1 scratch
            tc.tile_pool(name="bi", bufs=2) as bip,
            tc.tile_pool(name="work", bufs=2) as workp,
            tc.tile_pool(name="small", bufs=3) as smallp,
            tc.tile_pool(name="const", bufs=1) as constp,
            tc.tile_pool(name="ps_s", bufs=3, space="PSUM") as ps_sp,
            tc.tile_pool(name="ps_bc", bufs=2, space="PSUM") as ps_bcp,
            tc.tile_pool(name="ps_v", bufs=2, space="PSUM") as ps_vp,
            tc.tile_pool(name="ps_w", bufs=1, space="PSUM") as ps_wp,
        ):
            # ---- persistent state ----
            Xb = big.tile([P, S * SW], BF16)          # bf16 input, (s, j, o, d)
            vb_all = big.tile([P, S * OD], BF16)      # v bcast to all partitions
            b_cum = big.tile([P, S * NJ * O], F32)    # routing logits (s, j, o)
            c_t = big.tile([P, S * NJ * O], BF16)     # softmax coeffs (s, j, o)

            # ---- constants ----
            mask128 = constp.tile([P, OD], F32)       # 4 strips of block-diag o/od
            ones16 = constp.tile([1, D], F32)
            onesrow = constp.tile([32, P], BF16)      # src for A strips
            A = constp.tile([P, TB * P], BF16)        # A[:, t*P:(t+1)*P]: ones on strip t
            c01 = constp.tile([P, O], BF16)           # uniform routing c = 0.1

            nc.vector.memset(ones16[:], 1.0)
            nc.vector.memset(mask128[:], 0.0)
            for t in range(TB):
                for o in range(O):
                    # engines can't write at partition offsets; DMA can
                    nc.sync.dma_start(
                        mask128[32 * t + o : 32 * t + o + 1, o * D : (o + 1) * D],
                        ones16[:],
                    )
            nc.vector.memset(onesrow[:], 1.0)
            nc.vector.memset(A[:], 0.0)
            for t in range(TB):
                nc.sync.dma_start(
                    A[32 * t : 32 * (t + 1), t * P : (t + 1) * P], onesrow[:]
                )
            nc.vector.memset(c01[:], 0.1)

            # PE warmup so PE observes the DVE-const tick once
            warm = ps_wp.tile([1, O], F32, tag="warm")
            nc.tensor.matmul(warm[:], A[:, :1], c01[:], start=True, stop=True)

            def load_batch(q):
                for t in range(TB):
                    b = TB * q + t
                    st = stagep.tile([P, SW], F32)
                    nc.sync.dma_start(
                        st[:],
                        x[b].rearrange("(p j) o d -> p (j o d)", p=P),
                    )
                    nc.vector.tensor_copy(Xb[:, b * SW : (b + 1) * SW], st[:])

            def einsum1_batch(q, k):
                """b_cum[s,i,o] += sum_d X * v for samples 4q..4q+3."""
                on_gps = q in GPS_BATCHES
                eng = nc.gpsimd if on_gps else nc.vector
                sfx = "g" if on_gps else "d"
                s0 = TB * q
                M = TB * NJ * O          # 360 d-groups
                tmp = e1p.tile([P, TB * SW], BF16, tag="tmp" + sfx)
                xs = Xb[:, s0 * SW : (s0 + TB) * SW].rearrange(
                    "p (s j f) -> p s j f", s=TB, j=NJ
                )
                vb = vb_all[:, s0 * OD : (s0 + TB) * OD].rearrange(
                    "p (s f) -> p s f", s=TB
                )
                eng.tensor_mul(
                    tmp[:].rearrange("p (s j f) -> p s j f", s=TB, j=NJ),
                    xs,
                    _rep_mid(vb, NJ, at=2),
                )
                tv = tmp[:].rearrange("p (m e) -> p m e", e=D)
                t1 = e1p.tile([P, M * 8], BF16, tag="t1" + sfx)
                t1v = t1[:].rearrange("p (m e) -> p m e", e=8)
                eng.tensor_add(t1v, tv[:, :, 0:8], tv[:, :, 8:16])
                t2 = e1p.tile([P, M * 4], BF16, tag="t2" + sfx)
                t2v = t2[:].rearrange("p (m e) -> p m e", e=4)
                eng.tensor_add(t2v, t1v[:, :, 0:4], t1v[:, :, 4:8])
                t3 = e1p.tile([P, M * 2], BF16, tag="t3" + sfx)
                t3v = t3[:].rearrange("p (m e) -> p m e", e=2)
                eng.tensor_add(t3v, t2v[:, :, 0:2], t2v[:, :, 2:4])
                bsl = b_cum[:, s0 * NJ * O : (s0 + TB) * NJ * O]
                if k == 0:
                    eng.tensor_add(
                        bsl.rearrange("p (m e) -> p m e", e=1),
                        t3v[:, :, 0:1],
                        t3v[:, :, 1:2],
                    )
                else:
                    binc = bip.tile([P, M], F32, tag="bi" + sfx)
                    eng.tensor_add(
                        binc[:].rearrange("p (m e) -> p m e", e=1),
                        t3v[:, :, 0:1],
                        t3v[:, :, 1:2],
                    )
                    nc.gpsimd.tensor_add(bsl, bsl, binc[:])

            def softmax_batch(q):
                s0 = TB * q
                M = TB * NJ              # 36 softmax rows per partition
                bsl = b_cum[:, s0 * NJ * O : (s0 + TB) * NJ * O]
                e_g = workp.tile([P, TB * NJ * O], F32, tag="eg")
                nc.scalar.activation(e_g[:], bsl, AF.Exp)
                rs = smallp.tile([P, M], F32, tag="rs")
                nc.vector.reduce_sum(
                    rs[:], e_g[:].rearrange("p (m o) -> p m o", o=O), axis=AX
                )
                rr = smallp.tile([P, M], F32, tag="rr")
                nc.vector.reciprocal(rr[:], rs[:])
                nc.vector.tensor_mul(
                    c_t[:, s0 * NJ * O : (s0 + TB) * NJ * O].rearrange(
                        "p (m o) -> p m o", o=O
                    ),
                    e_g[:].rearrange("p (m o) -> p m o", o=O),
                    _rep_last(rr[:], O),
                )

            def einsum2_batch(q, stationary):
                """4 col-tiled samples' s = sum_i c[i,o] X[i,od] -> ps [128,160]."""
                ps = ps_sp.tile([P, OD], F32)
                for j in range(NJ):
                    for t in range(TB):
                        b = TB * q + t
                        if stationary is None:
                            lhsT = c_t[:, (b * NJ + j) * O : (b * NJ + j + 1) * O]
                        else:
                            lhsT = stationary[:]
                        nc.tensor.matmul(
                            ps[32 * t : 32 * t + O, :],
                            lhsT,
                            Xb[:, (b * SW + j * OD) : (b * SW + (j + 1) * OD)],
                            start=(j == 0),
                            stop=(j == NJ - 1),
                            tile_position=(0, 32 * t),
                        )
                return ps

            def squash_start(q, ps):
                """masked s + ACT square/accum -> (masked, l2)."""
                masked = smallp.tile([P, OD], F32, tag="mk")
                nc.vector.tensor_mul(masked[:], ps[:], mask128[:])
                sq = smallp.tile([P, OD], BF16, tag="sq")
                l2 = smallp.tile([P, 1], F32, tag="l2")
                nc.scalar.activation(sq[:], masked[:], AF.Square, accum_out=l2[:])
                return masked, l2

            def squash_finish(q, masked, l2):
                """scale = sqrt(l2)/(1+l2) via DVE bit-trick sqrt + one Newton
                step (keeps ACT on the exp table set; no table reloads)."""
                half_i = smallp.tile([P, 1], F32, tag="hi")
                nc.vector.tensor_scalar(
                    half_i[:].bitcast(mybir.dt.int32),
                    l2[:].bitcast(mybir.dt.int32),
                    1,
                    None,
                    op0=ALU.logical_shift_right,
                )
                rt0 = smallp.tile([P, 1], F32, tag="rt")
                nc.vector.tensor_scalar(
                    rt0[:].bitcast(mybir.dt.int32),
                    half_i[:].bitcast(mybir.dt.int32),
                    0x1FBD1DF5,
                    None,
                    op0=ALU.add,
                )
                q0 = smallp.tile([P, 1], F32, tag="q0")
                nc.vector.reciprocal(q0[:], rt0[:])
                t = smallp.tile([P, 1], F32, tag="t")
                nc.vector.tensor_mul(t[:], l2[:], q0[:])
                num = smallp.tile([P, 1], F32, tag="nm")
                nc.vector.tensor_add(num[:], rt0[:], t[:])
                den2 = smallp.tile([P, 1], F32, tag="d2")
                nc.vector.tensor_scalar(
                    den2[:], l2[:], 2.0, 2.0, op0=ALU.mult, op1=ALU.add
                )
                rden2 = smallp.tile([P, 1], F32, tag="rd")
                nc.vector.reciprocal(rden2[:], den2[:])
                sc = smallp.tile([P, 1], F32, tag="sc")
                nc.vector.tensor_mul(sc[:], num[:], rden2[:])
                v_full = smallp.tile([P, OD], BF16, tag="vf")
                nc.vector.tensor_single_scalar(v_full[:], masked[:], sc[:], op=ALU.mult)
                return v_full

            def broadcast_batch(q, v_full):
                """v per sample -> all partitions of vb_all (collapse+bcast matmul)."""
                for half in range(2):
                    bc = ps_bcp.tile([P, 2 * OD], F32, tag="bc")
                    for u in range(2):
                        t = 2 * half + u
                        nc.tensor.matmul(
                            bc[:, u * OD : (u + 1) * OD],
                            A[:, t * P : (t + 1) * P],
                            v_full[:],
                            start=True,
                            stop=True,
                        )
                    nc.vector.tensor_copy(
                        vb_all[
                            :,
                            (TB * q + 2 * half) * OD : (TB * q + 2 * half + 2) * OD,
                        ],
                        bc[:],
                    )

            def output_batch(q, v_full):
                for t in range(TB):
                    b = TB * q + t
                    psv = ps_vp.tile([1, OD], F32, tag="pv")
                    nc.tensor.matmul(
                        psv[:], A[:, t * P : t * P + 1], v_full[:],
                        start=True, stop=True,
                    )
                    v_sb = smallp.tile([1, OD], F32, tag="vo")
                    nc.vector.tensor_copy(v_sb[:], psv[:])
                    nc.sync.dma_start(
                        vout[b : b + 1].rearrange("b o d -> b (o d)"), v_sb[:]
                    )

            # Deferred squash finishes: software pipeline so the DVE fills the
            # ACT-square latency of batch q with batch q+1's einsum1 work.
            pending = []

            def flush_pending():
                while pending:
                    fq, fm, fl2, flast = pending.pop(0)
                    vf = squash_finish(fq, fm, fl2)
                    if flast:
                        output_batch(fq, vf)
                    else:
                        broadcast_batch(fq, vf)

            # ---- phase -1 (load + s0) merged with iteration 0 ----
            for q in range(NB):
                load_batch(q)
                ps = einsum2_batch(q, c01)
                m, l2 = squash_start(q, ps)
                vf = squash_finish(q, m, l2)
                broadcast_batch(q, vf)
                flush_pending()
                einsum1_batch(q, 0)
                softmax_batch(q)
                ps = einsum2_batch(q, None)
                m, l2 = squash_start(q, ps)
                pending.append((q, m, l2, False))
            flush_pending()

            # ---- iterations 1..2 ----
            for k in range(1, N_ITER):
                last = k == N_ITER - 1
                for q in range(NB):
                    einsum1_batch(q, k)
                    softmax_batch(q)
                    ps = einsum2_batch(q, None)
                    flush_pending()
                    m, l2 = squash_start(q, ps)
                    pending.append((q, m, l2, last))
                flush_pending()

    nc.compile()
    return nc


_cached = {}


def _get_nc():
    if "nc" not in _cached:
        _cached["nc"] = _build()
    return _cached["nc"]


def kernel(input, _trace=False):
    from concourse.bass_utils import run_bass_kernel_spmd

    input = np.ascontiguousarray(np.asarray(input, dtype=np.float32))
    assert input.shape == (B, I, O, D)
    nc = _get_nc()
    in_maps = [{"x": input[c * S : (c + 1) * S]} for c in range(NCORES)]
    res = run_bass_kernel_spmd(
        nc, in_maps, core_ids=list(range(NCORES)), trace=_trace
    )
    out = np.concatenate([r["v"] for r in res.results], axis=0)
    if _trace:
        kernel.last_exec_time_ns = res.exec_time_ns
        kernel.last_res = res
    return out.astype(np.float32)


kernel.last_exec_time_ns = None


# revision 23
# speedup vs baseline: 16.0441x; 1.3539x over previous
"""AgreementRouting (CapsNet dynamic routing) Trainium2 Bass kernel, v2.

Full input [256, 1152, 10, 16] f32 -> v [256, 10, 16] f32.
Data-parallel over batch: 32 samples per core on 8 cores.

Per-core plan (memory-regime):
  - Load: per-sample HWDGE DMA of contiguous f32 (i permuted so each
    partition reads ONE contiguous 5760B run: i = 9*p + j), staged in SBUF
    f32, converted to resident bf16 Xb by DVE 2x copies.  This replaces the
    old 2-byte-strided bf16 extraction that generated 5.9M descriptors.
  - Phase -1 (s0): einsum2 with constant c=0.1 stationary, col-tiled
    4 samples/PSUM tile, batched squash, matmul collapse+broadcast of v.
  - 3 routing iterations fully on-chip, batches of 4 samples:
      einsum1: bf16 mul (DVE 2x / GpSimd) + bf16 pair-tree reduction over d
               (TT adds at 2x beat tensor_reduce's 1x mode); 2 of 8 batches
               run on GpSimd to unload DVE (the bottleneck engine)
      softmax over o: ACT Exp + DVE sum/recip/mul
      einsum2: PE matmuls, c stationary, 4 samples col-tiled per PSUM tile
      squash: batched over 4 samples on [128,*] tiles; sqrt via exp(.5*ln)
              so ACT stays on the natural_log_exp table set (no reloads)
      v feedback: one [128,128]-ones matmul per sample does collapse AND
              broadcast-to-all-partitions in one shot.
  - All PE dependencies are DVE-produced (PE instrs carry one sem wait).
"""

import numpy as np

import concourse.bacc as bacc
import concourse.bass as bass
import concourse.mybir as mybir
import concourse.tile as tile
from concourse.bass import AP

NCORES = 8
B = 256
S = B // NCORES          # 32 samples per core
I = 1152
O = 10
D = 16
OD = O * D               # 160
P = 128
NJ = I // P              # 9: i = 9*p + j
SW = NJ * OD             # 1440 elems per sample per partition
TB = 4                   # batch: einsum1 group == einsum2 col-tiled samples
NB = S // TB             # 8 batches
GPS_BATCHES = ()         # GpSimd shares DVE's SBUF port: co-running 2-port
                         # DVE ops with GpSimd slows BOTH ~3.5x (measured),
                         # so einsum1 stays entirely on DVE
N_ITER = 3

F32 = mybir.dt.float32
BF16 = mybir.dt.bfloat16
AX = mybir.AxisListType.X
AF = mybir.ActivationFunctionType
ALU = mybir.AluOpType


def _rep_mid(a, n, at=1):
    """Insert a step-0 (broadcast) dim of size n into free dims at position `at`."""
    ap = [list(e) for e in a.ap]
    ap = ap[:at] + [[0, n]] + ap[at:]
    return AP(a.tensor, a.offset, ap)


def _rep_last(a, n):
    ap = [list(e) for e in a.ap] + [[0, n]]
    return AP(a.tensor, a.offset, ap)


def _build():
    nc = bacc.Bacc(None, target_bir_lowering=False)
    x = nc.dram_tensor("x", [S, I, O, D], F32, kind="ExternalInput")
    vout = nc.dram_tensor("v", [S, O, D], F32, kind="ExternalOutput")

    with tile.TileContext(nc) as tc:
        with (
            tc.tile_pool(name="big", bufs=1) as big,
            tc.tile_pool(name="stage", bufs=2) as stagep,
            tc.tile_pool(name="e1", bufs=1) as e1p,      # einsum1 scratch
            tc.tile_pool(name="bi", bufs=2) as bip,
            tc.tile_pool(name="work", bufs=2) as workp,
            tc.tile_pool(name="small", bufs=3) as smallp,
            tc.tile_pool(name="const", bufs=1) as constp,
            tc.tile_pool(name="ps_s", bufs=3, space="PSUM") as ps_sp,
            tc.tile_pool(name="ps_bc", bufs=2, space="PSUM") as ps_bcp,
            tc.tile_pool(name="ps_v", bufs=2, space="PSUM") as ps_vp,
            tc.tile_pool(name="ps_w", bufs=1, space="PSUM") as ps_wp,
        ):
            # ---- persistent state ----
            Xb = big.tile([P, S * SW], BF16)          # bf16 input, (s, j, o, d)
            vb_all = big.tile([P, S * OD], BF16)      # v bcast to all partitions
            b_cum = big.tile([P, S * NJ * O], F32)    # routing logits (s, j, o)
            c_t = big.tile([P, S * NJ * O], BF16)     # softmax coeffs (s, j, o)

            # ---- constants ----
            mask128 = constp.tile([P, OD], F32)       # 4 strips of block-diag o/od
            ones16 = constp.tile([1, D], F32)
            onesrow = constp.tile([32, P], BF16)      # src for A strips
            A = constp.tile([P, TB * P], BF16)        # A[:, t*P:(t+1)*P]: ones on strip t
            c01 = constp.tile([P, O], BF16)           # uniform routing c = 0.1

            nc.vector.memset(ones16[:], 1.0)
            nc.vector.memset(mask128[:], 0.0)
            for t in range(TB):
                for o in range(O):
                    # engines can't write at partition offsets; DMA can
                    nc.sync.dma_start(
                        mask128[32 * t + o : 32 * t + o + 1, o * D : (o + 1) * D],
                        ones16[:],
                    )
            nc.vector.memset(onesrow[:], 1.0)
            nc.vector.memset(A[:], 0.0)
            for t in range(TB):
                nc.sync.dma_start(
                    A[32 * t : 32 * (t + 1), t * P : (t + 1) * P], onesrow[:]
                )
            nc.vector.memset(c01[:], 0.1)

            # PE warmup so PE observes the DVE-const tick once
            warm = ps_wp.tile([1, O], F32, tag="warm")
            nc.tensor.matmul(warm[:], A[:, :1], c01[:], start=True, stop=True)

            def load_batch(q):
                b0 = TB * q
                st = stagep.tile([P, TB * SW], F32)
                nc.sync.dma_start(
                    st[:].rearrange("p (s f) -> p s f", s=TB),
                    x[b0 : b0 + TB].rearrange("s (p j) o d -> p s (j o d)", p=P),
                )
                nc.vector.tensor_copy(Xb[:, b0 * SW : (b0 + TB) * SW], st[:])

            def einsum1_batch(q, k):
                """b_cum[s,i,o] += sum_d X * v for samples 4q..4q+3."""
                on_gps = q in GPS_BATCHES
                eng = nc.gpsimd if on_gps else nc.vector
                sfx = "g" if on_gps else "d"
                s0 = TB * q
                M = TB * NJ * O          # 360 d-groups
                tmp = e1p.tile([P, TB * SW], BF16, tag="tmp" + sfx)
                xs = Xb[:, s0 * SW : (s0 + TB) * SW].rearrange(
                    "p (s j f) -> p s j f", s=TB, j=NJ
                )
                vb = vb_all[:, s0 * OD : (s0 + TB) * OD].rearrange(
                    "p (s f) -> p s f", s=TB
                )
                eng.tensor_mul(
                    tmp[:].rearrange("p (s j f) -> p s j f", s=TB, j=NJ),
                    xs,
                    _rep_mid(vb, NJ, at=2),
                )
                tv = tmp[:].rearrange("p (m e) -> p m e", e=D)
                t1 = e1p.tile([P, M * 8], BF16, tag="t1" + sfx)
                t1v = t1[:].rearrange("p (m e) -> p m e", e=8)
                eng.tensor_add(t1v, tv[:, :, 0:8], tv[:, :, 8:16])
                t2 = e1p.tile([P, M * 4], BF16, tag="t2" + sfx)
                t2v = t2[:].rearrange("p (m e) -> p m e", e=4)
                eng.tensor_add(t2v, t1v[:, :, 0:4], t1v[:, :, 4:8])
                t3 = e1p.tile([P, M * 2], BF16, tag="t3" + sfx)
                t3v = t3[:].rearrange("p (m e) -> p m e", e=2)
                eng.tensor_add(t3v, t2v[:, :, 0:2], t2v[:, :, 2:4])
                bsl = b_cum[:, s0 * NJ * O : (s0 + TB) * NJ * O]
                if k == 0:
                    eng.tensor_add(
                        bsl.rearrange("p (m e) -> p m e", e=1),
                        t3v[:, :, 0:1],
                        t3v[:, :, 1:2],
                    )
                else:
                    binc = bip.tile([P, M], F32, tag="bi" + sfx)
                    eng.tensor_add(
                        binc[:].rearrange("p (m e) -> p m e", e=1),
                        t3v[:, :, 0:1],
                        t3v[:, :, 1:2],
                    )
                    nc.vector.tensor_add(bsl, bsl, binc[:])

            def softmax_batch(q):
                s0 = TB * q
                M = TB * NJ              # 36 softmax rows per partition
                bsl = b_cum[:, s0 * NJ * O : (s0 + TB) * NJ * O]
                e_g = workp.tile([P, TB * NJ * O], F32, tag="eg")
                nc.scalar.activation(e_g[:], bsl, AF.Exp)
                rs = smallp.tile([P, M], F32, tag="rs")
                nc.vector.reduce_sum(
                    rs[:], e_g[:].rearrange("p (m o) -> p m o", o=O), axis=AX
                )
                rr = smallp.tile([P, M], F32, tag="rr")
                nc.vector.reciprocal(rr[:], rs[:])
                nc.vector.tensor_mul(
                    c_t[:, s0 * NJ * O : (s0 + TB) * NJ * O].rearrange(
                        "p (m o) -> p m o", o=O
                    ),
                    e_g[:].rearrange("p (m o) -> p m o", o=O),
                    _rep_last(rr[:], O),
                )

            def einsum2_batch(q, stationary):
                """4 col-tiled samples' s = sum_i c[i,o] X[i,od] -> ps [128,160]."""
                ps = ps_sp.tile([P, OD], F32)
                for j in range(NJ):
                    for t in range(TB):
                        b = TB * q + t
                        if stationary is None:
                            lhsT = c_t[:, (b * NJ + j) * O : (b * NJ + j + 1) * O]
                        else:
                            lhsT = stationary[:]
                        nc.tensor.matmul(
                            ps[32 * t : 32 * t + O, :],
                            lhsT,
                            Xb[:, (b * SW + j * OD) : (b * SW + (j + 1) * OD)],
                            start=(j == 0),
                            stop=(j == NJ - 1),
                            tile_position=(0, 32 * t),
                        )
                return ps

            def squash_start(q, ps):
                """masked s + ACT square/accum -> (masked, l2)."""
                masked = smallp.tile([P, OD], F32, tag="mk")
                nc.vector.tensor_mul(masked[:], ps[:], mask128[:])
                sq = smallp.tile([P, OD], BF16, tag="sq")
                l2 = smallp.tile([P, 1], F32, tag="l2")
                nc.scalar.activation(sq[:], masked[:], AF.Square, accum_out=l2[:])
                return masked, l2

            def squash_finish(q, masked, l2):
                """scale = sqrt(l2)/(1+l2) via DVE bit-trick sqrt + one Newton
                step (keeps ACT on the exp table set; no table reloads)."""
                half_i = smallp.tile([P, 1], F32, tag="hi")
                nc.vector.tensor_scalar(
                    half_i[:].bitcast(mybir.dt.int32),
                    l2[:].bitcast(mybir.dt.int32),
                    1,
                    None,
                    op0=ALU.logical_shift_right,
                )
                rt0 = smallp.tile([P, 1], F32, tag="rt")
                nc.vector.tensor_scalar(
                    rt0[:].bitcast(mybir.dt.int32),
                    half_i[:].bitcast(mybir.dt.int32),
                    0x1FBD1DF5,
                    None,
                    op0=ALU.add,
                )
                q0 = smallp.tile([P, 1], F32, tag="q0")
                nc.vector.reciprocal(q0[:], rt0[:])
                t = smallp.tile([P, 1], F32, tag="t")
                nc.vector.tensor_mul(t[:], l2[:], q0[:])
                num = smallp.tile([P, 1], F32, tag="nm")
                nc.vector.tensor_add(num[:], rt0[:], t[:])
                den2 = smallp.tile([P, 1], F32, tag="d2")
                nc.vector.tensor_scalar(
                    den2[:], l2[:], 2.0, 2.0, op0=ALU.mult, op1=ALU.add
                )
                rden2 = smallp.tile([P, 1], F32, tag="rd")
                nc.vector.reciprocal(rden2[:], den2[:])
                sc = smallp.tile([P, 1], F32, tag="sc")
                nc.vector.tensor_mul(sc[:], num[:], rden2[:])
                v_full = smallp.tile([P, OD], BF16, tag="vf")
                nc.vector.tensor_single_scalar(v_full[:], masked[:], sc[:], op=ALU.mult)
                return v_full

            def broadcast_batch(q, v_full):
                """v per sample -> all partitions of vb_all (collapse+bcast matmul)."""
                for half in range(2):
                    bc = ps_bcp.tile([P, 2 * OD], F32, tag="bc")
                    for u in range(2):
                        t = 2 * half + u
                        nc.tensor.matmul(
                            bc[:, u * OD : (u + 1) * OD],
                            A[:, t * P : (t + 1) * P],
                            v_full[:],
                            start=True,
                            stop=True,
                        )
                    nc.scalar.copy(
                        vb_all[
                            :,
                            (TB * q + 2 * half) * OD : (TB * q + 2 * half + 2) * OD,
                        ],
                        bc[:],
                    )

            def output_batch(q, v_full):
                for t in range(TB):
                    b = TB * q + t
                    psv = ps_vp.tile([1, OD], F32, tag="pv")
                    nc.tensor.matmul(
                        psv[:], A[:, t * P : t * P + 1], v_full[:],
                        start=True, stop=True,
                    )
                    v_sb = smallp.tile([1, OD], F32, tag="vo")
                    nc.scalar.copy(v_sb[:], psv[:])
                    nc.sync.dma_start(
                        vout[b : b + 1].rearrange("b o d -> b (o d)"), v_sb[:]
                    )

            # Deferred squash finishes: software pipeline so the DVE fills the
            # ACT-square latency of batch q with batch q+1's einsum1 work.
            pending = []

            def flush_pending():
                while pending:
                    fq, fm, fl2, flast = pending.pop(0)
                    vf = squash_finish(fq, fm, fl2)
                    if flast:
                        output_batch(fq, vf)
                    else:
                        broadcast_batch(fq, vf)

            # ---- phase -1 (load + s0) merged with iteration 0 ----
            for q in range(NB):
                load_batch(q)
                ps = einsum2_batch(q, c01)
                m, l2 = squash_start(q, ps)
                vf = squash_finish(q, m, l2)
                broadcast_batch(q, vf)
                flush_pending()
                einsum1_batch(q, 0)
                softmax_batch(q)
                ps = einsum2_batch(q, None)
                m, l2 = squash_start(q, ps)
                pending.append((q, m, l2, False))
            flush_pending()

            # ---- iterations 1..2 ----
            for k in range(1, N_ITER):
                last = k == N_ITER - 1
                for q in range(NB):
                    einsum1_batch(q, k)
                    softmax_batch(q)
                    ps = einsum2_batch(q, None)
                    flush_pending()
                    m, l2 = squash_start(q, ps)
                    pending.append((q, m, l2, last))
                flush_pending()

    nc.compile()
    return nc


_cached = {}


def _get_nc():
    if "nc" not in _cached:
        _cached["nc"] = _build()
    return _cached["nc"]


def kernel(input, _trace=False):
    from concourse.bass_utils import run_bass_kernel_spmd

    input = np.ascontiguousarray(np.asarray(input, dtype=np.float32))
    assert input.shape == (B, I, O, D)
    nc = _get_nc()
    in_maps = [{"x": input[c * S : (c + 1) * S]} for c in range(NCORES)]
    res = run_bass_kernel_spmd(
        nc, in_maps, core_ids=list(range(NCORES)), trace=_trace
    )
    out = np.concatenate([r["v"] for r in res.results], axis=0)
    if _trace:
        kernel.last_exec_time_ns = res.exec_time_ns
        kernel.last_res = res
    return out.astype(np.float32)


kernel.last_exec_time_ns = None
